# revision 7
# baseline (speedup 1.0000x reference)
"""Trainium2 Bass kernel for nn_LuongAttnDecoderRNN_79474074845199.

Strategy (8-core SPMD, pure data parallel per sharding hint):
  - Graphs (bs=256) are sharded 32/core; batch is sorted so each core owns a
    contiguous node range. Each graph's nodes are padded to a uniform L so one
    compiled program serves all cores (segment boundaries are compile-time).
  - Per-core batch order is ROTATED so that each core's own 32 graphs sit at
    rows 0..31 of the first half-tile; this keeps the program identical across
    cores (host un-rotates outputs).
  - The small dense net (enc -> GRU -> proj, compress -> out) runs replicated
    on every core in transposed space (activations stored as [feature, batch]).
  - scores = rnn_out @ nodes^T computed as [b, node] tiles on the PE
    (lhsT = rnn_outT, rhs = host-pre-transposed nodesT), softmax without
    max-subtraction (scores are bounded ~|30|), denominators via ACT exp
    accum_out per segment, exact zero-pad correction by subtracting pad counts.
  - Context pooling: PE-transpose of each core's own 32 e-rows -> [node, 32]
    chunks, masked by a host one-hot, then bf16 matmul against natural-layout
    bf16 nodes accumulating [32, 512] in PSUM; scaled by 1/denom diag.
"""

import numpy as np
import ml_dtypes

BS, N_NODES, IN, H, OUT = 256, 51200, 512, 512, 256
NCORES = 8
GPC = BS // NCORES  # graphs per core = 32

_PROG_CACHE = {}


def _round_f32r(x):
    """Round-to-nearest-even fp32 -> fp32r (11-bit mantissa, low 12 bits zero)."""
    u = np.ascontiguousarray(x, np.float32).view(np.uint32).astype(np.uint64)
    u = u + 0x7FF + ((u >> 12) & 1)
    u = (u & 0xFFFFF000).astype(np.uint32)
    return u.view(np.float32)


def _build_program(L, repeat=1):
    import concourse.bacc as bacc
    import concourse.tile as tile
    import concourse.mybir as mybir
    from contextlib import ExitStack

    F32 = mybir.dt.float32
    F32R = mybir.dt.float32r
    BF16 = mybir.dt.bfloat16
    AF = mybir.ActivationFunctionType
    ALU = mybir.AluOpType
    AX = mybir.AxisListType

    NPAD = GPC * L
    CHUNK = 4 * L                 # per-PSUM-tile columns (4 segments)
    NCHUNK = NPAD // CHUNK        # chunks per half
    NSUB = CHUNK // 128           # 128-node subchunks per chunk
    MMN = 512                     # matmul free-dim granularity
    assert CHUNK % MMN == 0

    nc = bacc.Bacc("TRN2", target_bir_lowering=False, debug=False)

    # ---- DRAM I/O ----
    ndT_d = nc.dram_tensor("ndT", [H, NPAD], F32R, kind="ExternalInput").ap()
    nat_d = nc.dram_tensor("nat", [NPAD, H], BF16, kind="ExternalInput").ap()
    oneh_d = nc.dram_tensor("oneh", [NPAD, GPC], F32, kind="ExternalInput").ap()
    inT_d = nc.dram_tensor("inT", [IN, BS], F32R, kind="ExternalInput").ap()
    lhT_d = nc.dram_tensor("lhT", [H, BS], F32R, kind="ExternalInput").ap()
    encw_d = nc.dram_tensor("encw", [IN, H], F32R, kind="ExternalInput").ap()
    wih_d = nc.dram_tensor("wih", [H, 3 * H], F32R, kind="ExternalInput").ap()
    whh_d = nc.dram_tensor("whh", [H, 3 * H], F32R, kind="ExternalInput").ap()
    projw_d = nc.dram_tensor("projw", [H, H], F32R, kind="ExternalInput").ap()
    cw_d = nc.dram_tensor("cw", [3 * H, H], F32, kind="ExternalInput").ap()
    ow_d = nc.dram_tensor("ow", [H, OUT], F32, kind="ExternalInput").ap()
    bias_d = nc.dram_tensor("bias", [128, 30], F32, kind="ExternalInput").ap()
    padc_d = nc.dram_tensor("padc", [128, GPC], F32, kind="ExternalInput").ap()
    i32_d = nc.dram_tensor("i32", [32, 32], F32, kind="ExternalInput").ap()

    attn_d = nc.dram_tensor("attn", [BS, NPAD], F32, kind="ExternalOutput").ap()
    hT_d = nc.dram_tensor("hT", [H, BS], F32, kind="ExternalOutput").ap()
    outT_d = nc.dram_tensor("outT", [OUT, GPC], F32, kind="ExternalOutput").ap()

    split_kp = lambda ap: ap.rearrange("(k p) n -> p k n", p=128)

    with tile.TileContext(nc) as tc:
      for _rep in range(repeat):
        with ExitStack() as octx:
            persist = octx.enter_context(tc.tile_pool(name="persist", bufs=1))
            bias_t = persist.tile([128, 30], F32, tag="bias")
            i32_t = persist.tile([32, 32], F32, tag="i32")
            padc_t = persist.tile([128, GPC], F32, tag="padc")
            oneh_t = persist.tile([128, NPAD // 128, GPC], F32, tag="oneh")
            xT = persist.tile([128, 4, BS], F32R, tag="xT")
            rT = persist.tile([128, 4, BS], F32R, tag="rT")
            cw_t = persist.tile([128, 12, H], F32, tag="cw")
            ow_t = persist.tile([128, 4, OUT], F32, tag="ow")
            den = [persist.tile([128, GPC], F32, tag=f"den{h}", name=f"den{h}")
                   for h in range(2)]
            rcp = [persist.tile([128, GPC], F32, tag=f"rcp{h}", name=f"rcp{h}")
                   for h in range(2)]
            ctxT = persist.tile([128, 4, GPC], F32, tag="ctxT")

            nc.sync.dma_start(bias_t[:], bias_d)
            nc.sync.dma_start(i32_t[:], i32_d)
            nc.sync.dma_start(padc_t[:], padc_d)
            nc.sync.dma_start(oneh_t[:], oneh_d.rearrange("(s p) j -> p s j", p=128))
            nc.sync.dma_start(cw_t[:], split_kp(cw_d))
            nc.sync.dma_start(ow_t[:], split_kp(ow_d))

            # ================= small net (replicated, transposed space) =====
            with ExitStack() as ctx:
                wp = ctx.enter_context(tc.tile_pool(name="wts", bufs=1))
                sn = ctx.enter_context(tc.tile_pool(name="sn", bufs=1))
                p1 = ctx.enter_context(tc.tile_pool(name="p1", bufs=2, space="PSUM"))

                encw_t = wp.tile([128, 4, H], F32R, tag="encw")
                wih_t = wp.tile([128, 4, 3 * H], F32R, tag="wih")
                whh_t = wp.tile([128, 4, 3 * H], F32R, tag="whh")
                projw_t = wp.tile([128, 4, H], F32R, tag="projw")
                inT_t = wp.tile([128, 4, BS], F32R, tag="inT")
                lhT_t = wp.tile([128, 4, BS], F32R, tag="lhT")
                nc.sync.dma_start(encw_t[:], split_kp(encw_d))
                nc.sync.dma_start(wih_t[:], split_kp(wih_d))
                nc.sync.dma_start(whh_t[:], split_kp(whh_d))
                nc.sync.dma_start(projw_t[:], split_kp(projw_d))
                nc.sync.dma_start(inT_t[:], split_kp(inT_d))
                nc.sync.dma_start(lhT_t[:], split_kp(lhT_d))

                r_t = sn.tile([128, 4, BS], F32, tag="r")
                z_t = sn.tile([128, 4, BS], F32, tag="z")
                n_t = sn.tile([128, 4, BS], F32, tag="n")
                hT_t = sn.tile([128, 4, BS], F32, tag="h")
                rh_t = sn.tile([128, 4, BS], F32R, tag="rh")

                # enc: xT = relu(enc_wT.T @ inT + b)
                for m in range(4):
                    ps = p1.tile([128, BS], F32)
                    for k in range(4):
                        nc.tensor.matmul(ps[:], encw_t[:, k, m * 128:(m + 1) * 128],
                                         inT_t[:, k, :], start=(k == 0), stop=(k == 3))
                    nc.scalar.activation(xT[:, m, :], ps[:], AF.Relu,
                                         bias=bias_t[:, m:m + 1])

                # r and z gates: sigmoid(W_ih.x + W_hh.h + b)  (PE accumulates both)
                for gi, bcol, dst in ((0, 4, r_t), (1, 8, z_t)):
                    for m in range(4):
                        g0 = gi * 512 + m * 128
                        ps = p1.tile([128, BS], F32)
                        for k in range(4):
                            nc.tensor.matmul(ps[:], wih_t[:, k, g0:g0 + 128],
                                             xT[:, k, :], start=(k == 0), stop=False)
                        for k in range(4):
                            nc.tensor.matmul(ps[:], whh_t[:, k, g0:g0 + 128],
                                             lhT_t[:, k, :], start=False, stop=(k == 3))
                        nc.scalar.activation(dst[:, m, :], ps[:], AF.Sigmoid,
                                             bias=bias_t[:, bcol + m:bcol + m + 1])

                # n gate: tanh(gi_n + r * gh_n + b)
                for m in range(4):
                    g0 = 2 * 512 + m * 128
                    ps_i = p1.tile([128, BS], F32, tag="psi")
                    ps_h = p1.tile([128, BS], F32, tag="psh")
                    for k in range(4):
                        nc.tensor.matmul(ps_i[:], wih_t[:, k, g0:g0 + 128],
                                         xT[:, k, :], start=(k == 0), stop=(k == 3))
                    for k in range(4):
                        nc.tensor.matmul(ps_h[:], whh_t[:, k, g0:g0 + 128],
                                         lhT_t[:, k, :], start=(k == 0), stop=(k == 3))
                    tmp0 = sn.tile([128, BS], F32, tag="tmp0")
                    nc.scalar.activation(tmp0[:], ps_h[:], AF.Identity,
                                         bias=bias_t[:, 26 + m:27 + m])
                    tmp = sn.tile([128, BS], F32, tag="tmp")
                    nc.vector.tensor_mul(tmp[:], r_t[:, m, :], tmp0[:])
                    tmp2 = sn.tile([128, BS], F32, tag="tmp2")
                    nc.vector.tensor_add(tmp2[:], tmp[:], ps_i[:])
                    nc.scalar.activation(n_t[:, m, :], tmp2[:], AF.Tanh,
                                         bias=bias_t[:, 12 + m:13 + m])

                # h' = n + z*(h - n);  rh = relu(h')
                for m in range(4):
                    d1 = sn.tile([128, BS], F32, tag="d1")
                    nc.vector.tensor_sub(d1[:], lhT_t[:, m, :].bitcast(F32), n_t[:, m, :])
                    d2 = sn.tile([128, BS], F32, tag="d2")
                    nc.vector.tensor_mul(d2[:], z_t[:, m, :], d1[:])
                    nc.vector.tensor_add(hT_t[:, m, :], n_t[:, m, :], d2[:])
                    nc.vector.tensor_scalar_max(rh_t[:, m, :], hT_t[:, m, :], 0.0)
                nc.sync.dma_start(split_kp(hT_d), hT_t[:])

                # proj: rnn_outT = projT.T @ rh + b   (emitted as f32r for scores)
                for m in range(4):
                    ps = p1.tile([128, BS], F32)
                    for k in range(4):
                        nc.tensor.matmul(ps[:], projw_t[:, k, m * 128:(m + 1) * 128],
                                         rh_t[:, k, :], start=(k == 0), stop=(k == 3))
                    nc.scalar.activation(rT[:, m, :], ps[:], AF.Identity,
                                         bias=bias_t[:, 16 + m:17 + m])

            # ================= scores / softmax / pooling ====================
            with ExitStack() as ctx:
                ep = ctx.enter_context(tc.tile_pool(name="e", bufs=1))
                ndp = ctx.enter_context(tc.tile_pool(name="nd", bufs=2))
                natp = ctx.enter_context(tc.tile_pool(name="nat", bufs=3))
                eselp = ctx.enter_context(tc.tile_pool(name="esel", bufs=3))
                scp = ctx.enter_context(tc.tile_pool(name="sc", bufs=2, space="PSUM"))
                tpp = ctx.enter_context(tc.tile_pool(name="tp", bufs=2, space="PSUM"))
                cxp = ctx.enter_context(tc.tile_pool(name="cx", bufs=1, space="PSUM"))
                fin = ctx.enter_context(tc.tile_pool(name="fin", bufs=1))
                p3 = ctx.enter_context(tc.tile_pool(name="p3", bufs=1, space="PSUM"))

                ctx_ps = cxp.tile([GPC, H], F32)
                e_tiles = [[ep.tile([128, 2 * CHUNK], F32, tag=f"e{h}_{ci}",
                                    name=f"e{h}_{ci}")
                            for ci in range(NCHUNK // 2)] for h in range(2)]

                nat_r = nat_d.rearrange("(a k p) f -> a p k f", k=4, p=128)
                sub = 0
                for ci in range(NCHUNK):
                    nd = ndp.tile([128, 4, CHUNK], F32R, tag="nd")
                    nc.sync.dma_start(
                        nd[:], split_kp(ndT_d)[:, :, ci * CHUNK:(ci + 1) * CHUNK])
                    nats = []
                    for g2 in range(CHUNK // 512):
                        natt = natp.tile([128, 4, H], BF16, tag="nat")
                        nc.sync.dma_start(natt[:], nat_r[ci * (CHUNK // 512) + g2])
                        nats.append(natt)
                    for half in range(2):
                        ps = scp.tile([128, CHUNK], F32)
                        for s2 in range(CHUNK // MMN):
                            for k in range(4):
                                nc.tensor.matmul(
                                    ps[:, s2 * MMN:(s2 + 1) * MMN],
                                    rT[:, k, half * 128:(half + 1) * 128],
                                    nd[:, k, s2 * MMN:(s2 + 1) * MMN],
                                    start=(k == 0), stop=(k == 3))
                        et = e_tiles[half][ci // 2]
                        e0 = (ci % 2) * CHUNK
                        for s in range(4):
                            seg = ci * 4 + s
                            nc.scalar.activation(
                                et[:, e0 + s * L:e0 + (s + 1) * L],
                                ps[:, s * L:(s + 1) * L],
                                AF.Exp, accum_out=den[half][:, seg:seg + 1])
                        if half == 0:
                            for j in range(NSUB):
                                gidx = sub + j
                                pst = tpp.tile([128, 32], F32)
                                nc.tensor.transpose(
                                    pst[:], et[0:32, e0 + j * 128:e0 + (j + 1) * 128],
                                    i32_t[:])
                                esel = eselp.tile([128, GPC], BF16, tag="esel")
                                nc.vector.tensor_mul(
                                    esel[:], oneh_t[:, gidx, :], pst[:])
                                nc.tensor.matmul(
                                    ctx_ps[:], esel[:], nats[j // 4][:, j % 4, :],
                                    start=(gidx == 0), stop=(gidx == NPAD // 128 - 1),
                                    skip_group_check=True)
                    sub += NSUB

                # denominators: subtract pad counts, reciprocal
                for half in range(2):
                    nc.vector.tensor_sub(den[half][:], den[half][:], padc_t[:])
                    nc.vector.reciprocal(rcp[half][:], den[half][:])

                # normalize + write attention rows
                for half in range(2):
                    for ci2 in range(NCHUNK // 2):
                        et = e_tiles[half][ci2]
                        for s in range(8):
                            seg = ci2 * 8 + s
                            nc.vector.tensor_scalar_mul(
                                et[:, s * L:(s + 1) * L], et[:, s * L:(s + 1) * L],
                                rcp[half][:, seg:seg + 1])
                        nc.sync.dma_start(
                            attn_d[half * 128:(half + 1) * 128,
                                   ci2 * 2 * CHUNK:(ci2 + 1) * 2 * CHUNK], et[:])

                # context = diag(1/den) * ctx_ps
                dtmp = fin.tile([32, 32], F32, tag="dtmp")
                nc.vector.tensor_mul(dtmp[:], rcp[0][0:32, :], i32_t[:])
                rdiag = fin.tile([32, 1], F32, tag="rdiag")
                nc.vector.tensor_reduce(rdiag[:], dtmp[:], axis=AX.X, op=ALU.add)
                ctx_sb = fin.tile([GPC, H], F32, tag="ctx")
                nc.vector.tensor_scalar_mul(ctx_sb[:], ctx_ps[:], rdiag[:])
                for m in range(4):
                    pst = tpp.tile([128, 32], F32)
                    nc.tensor.transpose(
                        pst[:], ctx_sb[:, m * 128:(m + 1) * 128], i32_t[:])
                    nc.vector.tensor_copy(ctxT[:, m, :], pst[:])

                # compress: relu(cw.T @ [rnn_out; context; x] + b)
                coT = fin.tile([128, 4, GPC], F32, tag="coT")
                for m in range(4):
                    ps = p3.tile([128, GPC], F32)
                    for kc in range(12):
                        if kc < 4:
                            rhs = rT[:, kc, 0:GPC].bitcast(F32)
                        elif kc < 8:
                            rhs = ctxT[:, kc - 4, :]
                        else:
                            rhs = xT[:, kc - 8, 0:GPC].bitcast(F32)
                        nc.tensor.matmul(ps[:], cw_t[:, kc, m * 128:(m + 1) * 128],
                                         rhs, start=(kc == 0), stop=(kc == 11))
                    nc.scalar.activation(coT[:, m, :], ps[:], AF.Relu,
                                         bias=bias_t[:, 20 + m:21 + m])

                # out layer
                ot = fin.tile([128, 2, GPC], F32, tag="ot")
                for m in range(2):
                    ps = p3.tile([128, GPC], F32)
                    for k in range(4):
                        nc.tensor.matmul(ps[:], ow_t[:, k, m * 128:(m + 1) * 128],
                                         coT[:, k, :], start=(k == 0), stop=(k == 3))
                    nc.scalar.activation(ot[:, m, :], ps[:], AF.Identity,
                                         bias=bias_t[:, 24 + m:25 + m])
                nc.sync.dma_start(
                    outT_d.rearrange("(m p) j -> p m j", p=128), ot[:])

    nc.compile()
    return nc


def _prep_host(inputs):
    """Shard + pad + transpose the full inputs into per-core input maps."""
    batch = np.asarray(inputs["batch"]).astype(np.int64)
    nodes = np.ascontiguousarray(np.asarray(inputs["nodes"], np.float32))
    cnt = np.bincount(batch, minlength=BS)
    assert cnt.sum() == batch.shape[0]
    starts = np.zeros(BS + 1, np.int64)
    np.cumsum(cnt, out=starts[1:])
    L = max(256, 128 * int(np.ceil(cnt.max() / 128)))
    NPAD = GPC * L

    inT = np.ascontiguousarray(np.asarray(inputs["input_seq"], np.float32)[0].T)
    lhT = np.ascontiguousarray(np.asarray(inputs["last_hidden"], np.float32)[0].T)
    W = {k: np.ascontiguousarray(np.asarray(inputs[k], np.float32).T)
         for k in ("enc_w", "W_ih", "W_hh", "proj_w", "compress_w", "out_w")}
    b_ih = np.asarray(inputs["b_ih"], np.float32)
    b_hh = np.asarray(inputs["b_hh"], np.float32)
    bias = np.zeros((128, 30), np.float32)
    bias[:, 0:4] = np.asarray(inputs["enc_b"], np.float32).reshape(4, 128).T
    bias[:, 4:12] = (b_ih + b_hh)[:1024].reshape(8, 128).T
    bias[:, 12:16] = b_ih[1024:].reshape(4, 128).T
    bias[:, 26:30] = b_hh[1024:].reshape(4, 128).T
    bias[:, 16:20] = np.asarray(inputs["proj_b"], np.float32).reshape(4, 128).T
    bias[:, 20:24] = np.asarray(inputs["compress_b"], np.float32).reshape(4, 128).T
    bias[:, 24:26] = np.asarray(inputs["out_b"], np.float32).reshape(2, 128).T
    i32 = np.eye(32, dtype=np.float32)

    in_maps = []
    meta = []
    for c in range(NCORES):
        ndT = np.zeros((H, NPAD), np.float32)
        nat = np.zeros((NPAD, H), np.float32)
        oneh = np.zeros((NPAD, GPC), np.float32)
        padc = np.zeros((128, GPC), np.float32)
        segs = []
        for j in range(GPC):
            g = c * GPC + j
            s, e = starts[g], starts[g + 1]
            n = e - s
            ndT[:, j * L:j * L + n] = nodes[s:e].T
            nat[j * L:j * L + n] = nodes[s:e]
            oneh[j * L:j * L + n, j] = 1.0
            padc[:, j] = L - n
            segs.append((s, e, n))
        meta.append(segs)
        in_maps.append({
            "ndT": _round_f32r(ndT),
            "nat": nat.astype(ml_dtypes.bfloat16),
            "oneh": oneh,
            "inT": _round_f32r(np.roll(inT, -GPC * c, axis=1)),
            "lhT": _round_f32r(np.roll(lhT, -GPC * c, axis=1)),
            "encw": _round_f32r(W["enc_w"]), "wih": _round_f32r(W["W_ih"]),
            "whh": _round_f32r(W["W_hh"]), "projw": _round_f32r(W["proj_w"]),
            "cw": W["compress_w"], "ow": W["out_w"],
            "bias": bias, "padc": padc, "i32": i32,
        })
    return in_maps, meta, L


def kernel(**inputs):
    from concourse.bass_utils import run_bass_kernel_spmd

    in_maps, meta, L = _prep_host(inputs)
    if L not in _PROG_CACHE:
        _PROG_CACHE[L] = _build_program(L)
    nc = _PROG_CACHE[L]

    res = run_bass_kernel_spmd(nc, in_maps, list(range(NCORES)))

    attn = np.empty((BS, N_NODES), np.float32)
    out = np.empty((BS, OUT), np.float32)
    for c in range(NCORES):
        r = res.results[c]
        ap = np.roll(r["attn"], GPC * c, axis=0)
        for j, (s, e, n) in enumerate(meta[c]):
            attn[:, s:e] = ap[:, j * L:j * L + n]
        out[c * GPC:(c + 1) * GPC] = r["outT"].T
    hidden = res.results[0]["hT"].T[None]
    return out, np.ascontiguousarray(hidden), attn


# revision 8
# speedup vs baseline: 9.2279x; 9.2279x over previous
"""Trainium2 Bass kernel for nn_LuongAttnDecoderRNN_79474074845199.

Strategy (8-core SPMD, pure data parallel per sharding hint):
  - Graphs (bs=256) are sharded 32/core; batch is sorted so each core owns a
    contiguous node range. Each graph's nodes are padded to a uniform L so one
    compiled program serves all cores (segment boundaries are compile-time).
  - Per-core batch order is ROTATED so that each core's own 32 graphs sit at
    rows 0..31 of the first half-tile; this keeps the program identical across
    cores (host un-rotates outputs).
  - The small dense net (enc -> GRU -> proj, compress -> out) runs replicated
    on every core in transposed space (activations stored as [feature, batch]).
  - scores = rnn_out @ nodes^T computed as [b, node] tiles on the PE
    (lhsT = rnn_outT, rhs = host-pre-transposed nodesT), softmax without
    max-subtraction (scores are bounded ~|30|), denominators via ACT exp
    accum_out per segment, exact zero-pad correction by subtracting pad counts.
  - Context pooling: PE-transpose of each core's own 32 e-rows -> [node, 32]
    chunks, masked by a host one-hot, then bf16 matmul against natural-layout
    bf16 nodes accumulating [32, 512] in PSUM; scaled by 1/denom diag.
"""

import numpy as np
import ml_dtypes

BS, N_NODES, IN, H, OUT = 256, 51200, 512, 512, 256
NCORES = 8
GPC = BS // NCORES  # graphs per core = 32

_PROG_CACHE = {}


def _round_f32r(x):
    """Round-to-nearest-even fp32 -> fp32r (11-bit mantissa, low 12 bits zero)."""
    u = np.ascontiguousarray(x, np.float32).view(np.uint32).astype(np.uint64)
    u = u + 0x7FF + ((u >> 12) & 1)
    u = (u & 0xFFFFF000).astype(np.uint32)
    return u.view(np.float32)


def _build_program(L, repeat=1):
    import concourse.bacc as bacc
    import concourse.tile as tile
    import concourse.mybir as mybir
    from contextlib import ExitStack

    F32 = mybir.dt.float32
    F32R = mybir.dt.float32r
    BF16 = mybir.dt.bfloat16
    AF = mybir.ActivationFunctionType
    ALU = mybir.AluOpType
    AX = mybir.AxisListType

    NPAD = GPC * L
    CHUNK = 4 * L                 # per-PSUM-tile columns (4 segments)
    NCHUNK = NPAD // CHUNK        # chunks per half
    NSUB = CHUNK // 128           # 128-node subchunks per chunk
    MMN = 512                     # matmul free-dim granularity
    assert CHUNK % MMN == 0

    nc = bacc.Bacc("TRN2", target_bir_lowering=False, debug=False)

    # ---- DRAM I/O ----
    ndT_d = nc.dram_tensor("ndT", [H, NPAD], F32R, kind="ExternalInput").ap()
    nat_d = nc.dram_tensor("nat", [NPAD, H], BF16, kind="ExternalInput").ap()
    oneh_d = nc.dram_tensor("oneh", [NPAD, GPC], F32, kind="ExternalInput").ap()
    inT_d = nc.dram_tensor("inT", [IN, BS], F32R, kind="ExternalInput").ap()
    lhT_d = nc.dram_tensor("lhT", [H, BS], F32R, kind="ExternalInput").ap()
    encw_d = nc.dram_tensor("encw", [IN, H], F32R, kind="ExternalInput").ap()
    wih_d = nc.dram_tensor("wih", [H, 3 * H], F32R, kind="ExternalInput").ap()
    whh_d = nc.dram_tensor("whh", [H, 3 * H], F32R, kind="ExternalInput").ap()
    projw_d = nc.dram_tensor("projw", [H, H], F32R, kind="ExternalInput").ap()
    cw_d = nc.dram_tensor("cw", [3 * H, H], F32, kind="ExternalInput").ap()
    ow_d = nc.dram_tensor("ow", [H, OUT], F32, kind="ExternalInput").ap()
    bias_d = nc.dram_tensor("bias", [128, 30], F32, kind="ExternalInput").ap()
    padc_d = nc.dram_tensor("padc", [128, GPC], F32, kind="ExternalInput").ap()
    i32_d = nc.dram_tensor("i32", [32, 32], F32, kind="ExternalInput").ap()

    attn_d = nc.dram_tensor("attn", [BS, NPAD], F32, kind="ExternalOutput").ap()
    hT_d = nc.dram_tensor("hT", [H, BS], F32, kind="ExternalOutput").ap()
    outT_d = nc.dram_tensor("outT", [OUT, GPC], F32, kind="ExternalOutput").ap()

    split_kp = lambda ap: ap.rearrange("(k p) n -> p k n", p=128)

    with tile.TileContext(nc) as tc:
      for _rep in range(repeat):
        with ExitStack() as octx:
            persist = octx.enter_context(tc.tile_pool(name="persist", bufs=1))
            bias_t = persist.tile([128, 30], F32, tag="bias")
            i32_t = persist.tile([32, 32], F32, tag="i32")
            padc_t = persist.tile([128, GPC], F32, tag="padc")
            oneh_t = persist.tile([128, NPAD // 128, GPC], F32, tag="oneh")
            xT = persist.tile([128, 4, BS], F32R, tag="xT")
            rT = persist.tile([128, 4, BS], F32R, tag="rT")
            cw_t = persist.tile([128, 12, H], F32, tag="cw")
            ow_t = persist.tile([128, 4, OUT], F32, tag="ow")
            den = [persist.tile([128, GPC], F32, tag=f"den{h}", name=f"den{h}")
                   for h in range(2)]
            rcp = [persist.tile([128, GPC], F32, tag=f"rcp{h}", name=f"rcp{h}")
                   for h in range(2)]
            ctxT = persist.tile([128, 4, GPC], F32, tag="ctxT")

            nc.sync.dma_start(bias_t[:], bias_d)
            nc.sync.dma_start(i32_t[:], i32_d)
            nc.sync.dma_start(padc_t[:], padc_d)
            nc.sync.dma_start(oneh_t[:], oneh_d.rearrange("(s p) j -> p s j", p=128))
            nc.sync.dma_start(cw_t[:], split_kp(cw_d))
            nc.sync.dma_start(ow_t[:], split_kp(ow_d))

            # ================= small net (replicated, transposed space) =====
            with ExitStack() as ctx:
                wp = ctx.enter_context(tc.tile_pool(name="wts", bufs=1))
                sn = ctx.enter_context(tc.tile_pool(name="sn", bufs=1))
                p1 = ctx.enter_context(tc.tile_pool(name="p1", bufs=2, space="PSUM"))

                encw_t = wp.tile([128, 4, H], F32R, tag="encw")
                wih_t = wp.tile([128, 4, 3 * H], F32R, tag="wih")
                whh_t = wp.tile([128, 4, 3 * H], F32R, tag="whh")
                projw_t = wp.tile([128, 4, H], F32R, tag="projw")
                inT_t = wp.tile([128, 4, BS], F32R, tag="inT")
                lhT_t = wp.tile([128, 4, BS], F32R, tag="lhT")
                nc.sync.dma_start(encw_t[:], split_kp(encw_d))
                nc.sync.dma_start(wih_t[:], split_kp(wih_d))
                nc.sync.dma_start(whh_t[:], split_kp(whh_d))
                nc.sync.dma_start(projw_t[:], split_kp(projw_d))
                nc.sync.dma_start(inT_t[:], split_kp(inT_d))
                nc.sync.dma_start(lhT_t[:], split_kp(lhT_d))

                r_t = sn.tile([128, 4, BS], F32, tag="r")
                z_t = sn.tile([128, 4, BS], F32, tag="z")
                n_t = sn.tile([128, 4, BS], F32, tag="n")
                hT_t = sn.tile([128, 4, BS], F32, tag="h")
                rh_t = sn.tile([128, 4, BS], F32R, tag="rh")

                # enc: xT = relu(enc_wT.T @ inT + b)
                for m in range(4):
                    ps = p1.tile([128, BS], F32)
                    for k in range(4):
                        nc.tensor.matmul(ps[:], encw_t[:, k, m * 128:(m + 1) * 128],
                                         inT_t[:, k, :], start=(k == 0), stop=(k == 3))
                    nc.scalar.activation(xT[:, m, :], ps[:], AF.Relu,
                                         bias=bias_t[:, m:m + 1])

                # r and z gates: sigmoid(W_ih.x + W_hh.h + b)  (PE accumulates both)
                for gi, bcol, dst in ((0, 4, r_t), (1, 8, z_t)):
                    for m in range(4):
                        g0 = gi * 512 + m * 128
                        ps = p1.tile([128, BS], F32)
                        for k in range(4):
                            nc.tensor.matmul(ps[:], wih_t[:, k, g0:g0 + 128],
                                             xT[:, k, :], start=(k == 0), stop=False)
                        for k in range(4):
                            nc.tensor.matmul(ps[:], whh_t[:, k, g0:g0 + 128],
                                             lhT_t[:, k, :], start=False, stop=(k == 3))
                        nc.scalar.activation(dst[:, m, :], ps[:], AF.Sigmoid,
                                             bias=bias_t[:, bcol + m:bcol + m + 1])

                # n gate: tanh(gi_n + r * gh_n + b)
                for m in range(4):
                    g0 = 2 * 512 + m * 128
                    ps_i = p1.tile([128, BS], F32, tag="psi")
                    ps_h = p1.tile([128, BS], F32, tag="psh")
                    for k in range(4):
                        nc.tensor.matmul(ps_i[:], wih_t[:, k, g0:g0 + 128],
                                         xT[:, k, :], start=(k == 0), stop=(k == 3))
                    for k in range(4):
                        nc.tensor.matmul(ps_h[:], whh_t[:, k, g0:g0 + 128],
                                         lhT_t[:, k, :], start=(k == 0), stop=(k == 3))
                    tmp0 = sn.tile([128, BS], F32, tag="tmp0")
                    nc.scalar.activation(tmp0[:], ps_h[:], AF.Identity,
                                         bias=bias_t[:, 26 + m:27 + m])
                    tmp = sn.tile([128, BS], F32, tag="tmp")
                    nc.vector.tensor_mul(tmp[:], r_t[:, m, :], tmp0[:])
                    tmp2 = sn.tile([128, BS], F32, tag="tmp2")
                    nc.vector.tensor_add(tmp2[:], tmp[:], ps_i[:])
                    nc.scalar.activation(n_t[:, m, :], tmp2[:], AF.Tanh,
                                         bias=bias_t[:, 12 + m:13 + m])

                # h' = n + z*(h - n);  rh = relu(h')
                for m in range(4):
                    d1 = sn.tile([128, BS], F32, tag="d1")
                    nc.vector.tensor_sub(d1[:], lhT_t[:, m, :].bitcast(F32), n_t[:, m, :])
                    d2 = sn.tile([128, BS], F32, tag="d2")
                    nc.vector.tensor_mul(d2[:], z_t[:, m, :], d1[:])
                    nc.vector.tensor_add(hT_t[:, m, :], n_t[:, m, :], d2[:])
                    nc.vector.tensor_scalar_max(rh_t[:, m, :], hT_t[:, m, :], 0.0)
                nc.sync.dma_start(split_kp(hT_d), hT_t[:])

                # proj: rnn_outT = projT.T @ rh + b   (emitted as f32r for scores)
                for m in range(4):
                    ps = p1.tile([128, BS], F32)
                    for k in range(4):
                        nc.tensor.matmul(ps[:], projw_t[:, k, m * 128:(m + 1) * 128],
                                         rh_t[:, k, :], start=(k == 0), stop=(k == 3))
                    nc.scalar.activation(rT[:, m, :], ps[:], AF.Identity,
                                         bias=bias_t[:, 16 + m:17 + m])

            # ================= scores / softmax / pooling ====================
            with ExitStack() as ctx:
                ep = ctx.enter_context(tc.tile_pool(name="e", bufs=1))
                ndp = ctx.enter_context(tc.tile_pool(name="nd", bufs=2))
                natp = ctx.enter_context(tc.tile_pool(name="nat", bufs=3))
                eselp = ctx.enter_context(tc.tile_pool(name="esel", bufs=3))
                scp = ctx.enter_context(tc.tile_pool(name="sc", bufs=2, space="PSUM"))
                tpp = ctx.enter_context(tc.tile_pool(name="tp", bufs=2, space="PSUM"))
                cxp = ctx.enter_context(tc.tile_pool(name="cx", bufs=1, space="PSUM"))
                fin = ctx.enter_context(tc.tile_pool(name="fin", bufs=1))
                p3 = ctx.enter_context(tc.tile_pool(name="p3", bufs=1, space="PSUM"))

                ctx_ps = cxp.tile([GPC, H], F32)
                e_tiles = [[ep.tile([128, 2 * CHUNK], F32, tag=f"e{h}_{ci}",
                                    name=f"e{h}_{ci}")
                            for ci in range(NCHUNK // 2)] for h in range(2)]

                nat_r = nat_d.rearrange("(a k p) f -> a p k f", k=4, p=128)
                sub = 0
                for ci in range(NCHUNK):
                    nd = ndp.tile([128, 4, CHUNK], F32R, tag="nd")
                    nc.sync.dma_start(
                        nd[:], split_kp(ndT_d)[:, :, ci * CHUNK:(ci + 1) * CHUNK])
                    nats = []
                    for g2 in range(CHUNK // 512):
                        natt = natp.tile([128, 4, H], BF16, tag="nat")
                        nc.sync.dma_start(natt[:], nat_r[ci * (CHUNK // 512) + g2])
                        nats.append(natt)
                    for half in range(2):
                        ps = scp.tile([128, CHUNK], F32)
                        for s2 in range(CHUNK // MMN):
                            for k in range(4):
                                nc.tensor.matmul(
                                    ps[:, s2 * MMN:(s2 + 1) * MMN],
                                    rT[:, k, half * 128:(half + 1) * 128],
                                    nd[:, k, s2 * MMN:(s2 + 1) * MMN],
                                    start=(k == 0), stop=(k == 3))
                        et = e_tiles[half][ci // 2]
                        e0 = (ci % 2) * CHUNK
                        for s in range(4):
                            seg = ci * 4 + s
                            nc.scalar.activation(
                                et[:, e0 + s * L:e0 + (s + 1) * L],
                                ps[:, s * L:(s + 1) * L],
                                AF.Exp, accum_out=den[half][:, seg:seg + 1])
                        if half == 0:
                            for j in range(NSUB):
                                gidx = sub + j
                                pst = tpp.tile([128, 32], F32)
                                nc.tensor.transpose(
                                    pst[:], et[0:32, e0 + j * 128:e0 + (j + 1) * 128],
                                    i32_t[:])
                                esel = eselp.tile([128, GPC], BF16, tag="esel")
                                nc.vector.tensor_mul(
                                    esel[:], oneh_t[:, gidx, :], pst[:])
                                nc.tensor.matmul(
                                    ctx_ps[:], esel[:], nats[j // 4][:, j % 4, :],
                                    start=(gidx == 0), stop=(gidx == NPAD // 128 - 1),
                                    skip_group_check=True)
                    # per-chunk: finalize denominators of these 4 segments,
                    # normalize in place, and drain finished 2-chunk tiles
                    for half in range(2):
                        s0 = ci * 4
                        nc.vector.tensor_sub(den[half][:, s0:s0 + 4],
                                             den[half][:, s0:s0 + 4],
                                             padc_t[:, s0:s0 + 4])
                        nc.vector.reciprocal(rcp[half][:, s0:s0 + 4],
                                             den[half][:, s0:s0 + 4])
                        et = e_tiles[half][ci // 2]
                        e0 = (ci % 2) * CHUNK
                        for s in range(4):
                            seg = s0 + s
                            nc.vector.tensor_scalar_mul(
                                et[:, e0 + s * L:e0 + (s + 1) * L],
                                et[:, e0 + s * L:e0 + (s + 1) * L],
                                rcp[half][:, seg:seg + 1])
                        if ci % 2 == 1:
                            ci2 = ci // 2
                            nc.sync.dma_start(
                                attn_d[half * 128:(half + 1) * 128,
                                       ci2 * 2 * CHUNK:(ci2 + 1) * 2 * CHUNK], et[:])
                    sub += NSUB

                # context = diag(1/den) * ctx_ps
                dtmp = fin.tile([32, 32], F32, tag="dtmp")
                nc.vector.tensor_mul(dtmp[:], rcp[0][0:32, :], i32_t[:])
                rdiag = fin.tile([32, 1], F32, tag="rdiag")
                nc.vector.tensor_reduce(rdiag[:], dtmp[:], axis=AX.X, op=ALU.add)
                ctx_sb = fin.tile([GPC, H], F32, tag="ctx")
                nc.vector.tensor_scalar_mul(ctx_sb[:], ctx_ps[:], rdiag[:])
                for m in range(4):
                    pst = tpp.tile([128, 32], F32)
                    nc.tensor.transpose(
                        pst[:], ctx_sb[:, m * 128:(m + 1) * 128], i32_t[:])
                    nc.vector.tensor_copy(ctxT[:, m, :], pst[:])

                # compress: relu(cw.T @ [rnn_out; context; x] + b)
                coT = fin.tile([128, 4, GPC], F32, tag="coT")
                for m in range(4):
                    ps = p3.tile([128, GPC], F32)
                    for kc in range(12):
                        if kc < 4:
                            rhs = rT[:, kc, 0:GPC].bitcast(F32)
                        elif kc < 8:
                            rhs = ctxT[:, kc - 4, :]
                        else:
                            rhs = xT[:, kc - 8, 0:GPC].bitcast(F32)
                        nc.tensor.matmul(ps[:], cw_t[:, kc, m * 128:(m + 1) * 128],
                                         rhs, start=(kc == 0), stop=(kc == 11))
                    nc.scalar.activation(coT[:, m, :], ps[:], AF.Relu,
                                         bias=bias_t[:, 20 + m:21 + m])

                # out layer
                ot = fin.tile([128, 2, GPC], F32, tag="ot")
                for m in range(2):
                    ps = p3.tile([128, GPC], F32)
                    for k in range(4):
                        nc.tensor.matmul(ps[:], ow_t[:, k, m * 128:(m + 1) * 128],
                                         coT[:, k, :], start=(k == 0), stop=(k == 3))
                    nc.scalar.activation(ot[:, m, :], ps[:], AF.Identity,
                                         bias=bias_t[:, 24 + m:25 + m])
                nc.sync.dma_start(
                    outT_d.rearrange("(m p) j -> p m j", p=128), ot[:])

    nc.compile()
    return nc


def _prep_host(inputs):
    """Shard + pad + transpose the full inputs into per-core input maps."""
    batch = np.asarray(inputs["batch"]).astype(np.int64)
    nodes = np.ascontiguousarray(np.asarray(inputs["nodes"], np.float32))
    cnt = np.bincount(batch, minlength=BS)
    assert cnt.sum() == batch.shape[0]
    starts = np.zeros(BS + 1, np.int64)
    np.cumsum(cnt, out=starts[1:])
    L = max(256, 128 * int(np.ceil(cnt.max() / 128)))
    NPAD = GPC * L

    inT = np.ascontiguousarray(np.asarray(inputs["input_seq"], np.float32)[0].T)
    lhT = np.ascontiguousarray(np.asarray(inputs["last_hidden"], np.float32)[0].T)
    W = {k: np.ascontiguousarray(np.asarray(inputs[k], np.float32).T)
         for k in ("enc_w", "W_ih", "W_hh", "proj_w", "compress_w", "out_w")}
    b_ih = np.asarray(inputs["b_ih"], np.float32)
    b_hh = np.asarray(inputs["b_hh"], np.float32)
    bias = np.zeros((128, 30), np.float32)
    bias[:, 0:4] = np.asarray(inputs["enc_b"], np.float32).reshape(4, 128).T
    bias[:, 4:12] = (b_ih + b_hh)[:1024].reshape(8, 128).T
    bias[:, 12:16] = b_ih[1024:].reshape(4, 128).T
    bias[:, 26:30] = b_hh[1024:].reshape(4, 128).T
    bias[:, 16:20] = np.asarray(inputs["proj_b"], np.float32).reshape(4, 128).T
    bias[:, 20:24] = np.asarray(inputs["compress_b"], np.float32).reshape(4, 128).T
    bias[:, 24:26] = np.asarray(inputs["out_b"], np.float32).reshape(2, 128).T
    i32 = np.eye(32, dtype=np.float32)

    in_maps = []
    meta = []
    for c in range(NCORES):
        ndT = np.zeros((H, NPAD), np.float32)
        nat = np.zeros((NPAD, H), np.float32)
        oneh = np.zeros((NPAD, GPC), np.float32)
        padc = np.zeros((128, GPC), np.float32)
        segs = []
        for j in range(GPC):
            g = c * GPC + j
            s, e = starts[g], starts[g + 1]
            n = e - s
            ndT[:, j * L:j * L + n] = nodes[s:e].T
            nat[j * L:j * L + n] = nodes[s:e]
            oneh[j * L:j * L + n, j] = 1.0
            padc[:, j] = L - n
            segs.append((s, e, n))
        meta.append(segs)
        in_maps.append({
            "ndT": _round_f32r(ndT),
            "nat": nat.astype(ml_dtypes.bfloat16),
            "oneh": oneh,
            "inT": _round_f32r(np.roll(inT, -GPC * c, axis=1)),
            "lhT": _round_f32r(np.roll(lhT, -GPC * c, axis=1)),
            "encw": _round_f32r(W["enc_w"]), "wih": _round_f32r(W["W_ih"]),
            "whh": _round_f32r(W["W_hh"]), "projw": _round_f32r(W["proj_w"]),
            "cw": W["compress_w"], "ow": W["out_w"],
            "bias": bias, "padc": padc, "i32": i32,
        })
    return in_maps, meta, L


def kernel(**inputs):
    from concourse.bass_utils import run_bass_kernel_spmd

    in_maps, meta, L = _prep_host(inputs)
    if L not in _PROG_CACHE:
        _PROG_CACHE[L] = _build_program(L)
    nc = _PROG_CACHE[L]

    res = run_bass_kernel_spmd(nc, in_maps, list(range(NCORES)))

    attn = np.empty((BS, N_NODES), np.float32)
    out = np.empty((BS, OUT), np.float32)
    for c in range(NCORES):
        r = res.results[c]
        ap = np.roll(r["attn"], GPC * c, axis=0)
        for j, (s, e, n) in enumerate(meta[c]):
            attn[:, s:e] = ap[:, j * L:j * L + n]
        out[c * GPC:(c + 1) * GPC] = r["outT"].T
    hidden = res.results[0]["hT"].T[None]
    return out, np.ascontiguousarray(hidden), attn


# revision 9
# speedup vs baseline: 14.8933x; 1.6139x over previous
"""Trainium2 Bass kernel for nn_LuongAttnDecoderRNN_79474074845199.

Strategy (8-core SPMD, pure data parallel per sharding hint):
  - Graphs (bs=256) are sharded 32/core; batch is sorted so each core owns a
    contiguous node range. Each graph's nodes are padded to a uniform L so one
    compiled program serves all cores (segment boundaries are compile-time).
  - Per-core batch order is ROTATED so that each core's own 32 graphs sit at
    rows 0..31 of the first half-tile; this keeps the program identical across
    cores (host un-rotates outputs).
  - The small dense net (enc -> GRU -> proj, compress -> out) runs replicated
    on every core in transposed space (activations stored as [feature, batch]).
  - scores = rnn_out @ nodes^T computed as [b, node] tiles on the PE
    (lhsT = rnn_outT, rhs = host-pre-transposed nodesT), softmax without
    max-subtraction (scores are bounded ~|30|), denominators via ACT exp
    accum_out per segment, exact zero-pad correction by subtracting pad counts.
  - Context pooling: PE-transpose of each core's own 32 e-rows -> [node, 32]
    chunks, masked by a host one-hot, then bf16 matmul against natural-layout
    bf16 nodes accumulating [32, 512] in PSUM; scaled by 1/denom diag.
"""

import numpy as np
import ml_dtypes

BS, N_NODES, IN, H, OUT = 256, 51200, 512, 512, 256
NCORES = 8
GPC = BS // NCORES  # graphs per core = 32

_PROG_CACHE = {}


def _round_f32r(x):
    """Round-to-nearest-even fp32 -> fp32r (11-bit mantissa, low 12 bits zero)."""
    u = np.ascontiguousarray(x, np.float32).view(np.uint32).astype(np.uint64)
    u = u + 0x7FF + ((u >> 12) & 1)
    u = (u & 0xFFFFF000).astype(np.uint32)
    return u.view(np.float32)


def _build_program(L, repeat=1):
    import concourse.bacc as bacc
    import concourse.tile as tile
    import concourse.mybir as mybir
    from contextlib import ExitStack

    F32 = mybir.dt.float32
    F32R = mybir.dt.float32r
    BF16 = mybir.dt.bfloat16
    AF = mybir.ActivationFunctionType
    ALU = mybir.AluOpType
    AX = mybir.AxisListType

    NPAD = GPC * L
    CHUNK = 4 * L                 # per-PSUM-tile columns (4 segments)
    NCHUNK = NPAD // CHUNK        # chunks per half
    NSUB = CHUNK // 128           # 128-node subchunks per chunk
    MMN = 512                     # matmul free-dim granularity
    assert CHUNK % MMN == 0

    nc = bacc.Bacc("TRN2", target_bir_lowering=False, debug=False)

    # ---- DRAM I/O ----
    ndT_d = nc.dram_tensor("ndT", [H, NPAD], F32R, kind="ExternalInput").ap()
    nat_d = nc.dram_tensor("nat", [NPAD, H], BF16, kind="ExternalInput").ap()
    oneh_d = nc.dram_tensor("oneh", [NPAD, GPC], F32, kind="ExternalInput").ap()
    inT_d = nc.dram_tensor("inT", [IN, BS], F32R, kind="ExternalInput").ap()
    lhT_d = nc.dram_tensor("lhT", [H, BS], F32R, kind="ExternalInput").ap()
    encw_d = nc.dram_tensor("encw", [IN, H], F32R, kind="ExternalInput").ap()
    wih_d = nc.dram_tensor("wih", [H, 3 * H], F32R, kind="ExternalInput").ap()
    whh_d = nc.dram_tensor("whh", [H, 3 * H], F32R, kind="ExternalInput").ap()
    projw_d = nc.dram_tensor("projw", [H, H], F32R, kind="ExternalInput").ap()
    cw_d = nc.dram_tensor("cw", [3 * H, H], F32, kind="ExternalInput").ap()
    ow_d = nc.dram_tensor("ow", [H, OUT], F32, kind="ExternalInput").ap()
    bias_d = nc.dram_tensor("bias", [128, 30], F32, kind="ExternalInput").ap()
    padc_d = nc.dram_tensor("padc", [128, GPC], F32, kind="ExternalInput").ap()
    i32_d = nc.dram_tensor("i32", [32, 32], F32, kind="ExternalInput").ap()

    attn_d = nc.dram_tensor("attn", [BS, NPAD], F32, kind="ExternalOutput").ap()
    hT_d = nc.dram_tensor("hT", [H, BS], F32, kind="ExternalOutput").ap()
    outT_d = nc.dram_tensor("outT", [OUT, GPC], F32, kind="ExternalOutput").ap()

    split_kp = lambda ap: ap.rearrange("(k p) n -> p k n", p=128)

    with tile.TileContext(nc) as tc:
      for _rep in range(repeat):
        with ExitStack() as octx:
            persist = octx.enter_context(tc.tile_pool(name="persist", bufs=1))
            bias_t = persist.tile([128, 30], F32, tag="bias")
            i32_t = persist.tile([32, 32], F32, tag="i32")
            padc_t = persist.tile([128, GPC], F32, tag="padc")
            oneh_t = persist.tile([128, NPAD // 128, GPC], F32, tag="oneh")
            xT = persist.tile([128, 4, BS], F32R, tag="xT")
            rT = persist.tile([128, 4, BS], F32R, tag="rT")
            cw_t = persist.tile([128, 12, H], F32, tag="cw")
            ow_t = persist.tile([128, 4, OUT], F32, tag="ow")
            den = [persist.tile([128, GPC], F32, tag=f"den{h}", name=f"den{h}")
                   for h in range(2)]
            rcp = [persist.tile([128, GPC], F32, tag=f"rcp{h}", name=f"rcp{h}")
                   for h in range(2)]
            ctxT = persist.tile([128, 4, GPC], F32, tag="ctxT")

            nc.sync.dma_start(bias_t[:], bias_d)
            nc.sync.dma_start(i32_t[:], i32_d)
            nc.sync.dma_start(padc_t[:], padc_d)
            nc.sync.dma_start(oneh_t[:], oneh_d.rearrange("(s p) j -> p s j", p=128))
            nc.sync.dma_start(cw_t[:], split_kp(cw_d))
            nc.sync.dma_start(ow_t[:], split_kp(ow_d))

            # ================= small net (replicated, transposed space) =====
            with ExitStack() as ctx:
                wp = ctx.enter_context(tc.tile_pool(name="wts", bufs=1))
                sn = ctx.enter_context(tc.tile_pool(name="sn", bufs=1))
                p1 = ctx.enter_context(tc.tile_pool(name="p1", bufs=2, space="PSUM"))

                encw_t = wp.tile([128, 4, H], F32R, tag="encw")
                wih_t = wp.tile([128, 4, 3 * H], F32R, tag="wih")
                whh_t = wp.tile([128, 4, 3 * H], F32R, tag="whh")
                projw_t = wp.tile([128, 4, H], F32R, tag="projw")
                inT_t = wp.tile([128, 4, BS], F32R, tag="inT")
                lhT_t = wp.tile([128, 4, BS], F32R, tag="lhT")
                nc.sync.dma_start(encw_t[:], split_kp(encw_d))
                nc.sync.dma_start(wih_t[:], split_kp(wih_d))
                nc.sync.dma_start(whh_t[:], split_kp(whh_d))
                nc.sync.dma_start(projw_t[:], split_kp(projw_d))
                nc.sync.dma_start(inT_t[:], split_kp(inT_d))
                nc.sync.dma_start(lhT_t[:], split_kp(lhT_d))

                r_t = sn.tile([128, 4, BS], F32, tag="r")
                z_t = sn.tile([128, 4, BS], F32, tag="z")
                n_t = sn.tile([128, 4, BS], F32, tag="n")
                hT_t = sn.tile([128, 4, BS], F32, tag="h")
                rh_t = sn.tile([128, 4, BS], F32R, tag="rh")

                # enc: xT = relu(enc_wT.T @ inT + b)
                for m in range(4):
                    ps = p1.tile([128, BS], F32)
                    for k in range(4):
                        nc.tensor.matmul(ps[:], encw_t[:, k, m * 128:(m + 1) * 128],
                                         inT_t[:, k, :], start=(k == 0), stop=(k == 3))
                    nc.scalar.activation(xT[:, m, :], ps[:], AF.Relu,
                                         bias=bias_t[:, m:m + 1])

                # r and z gates: sigmoid(W_ih.x + W_hh.h + b)  (PE accumulates both)
                for gi, bcol, dst in ((0, 4, r_t), (1, 8, z_t)):
                    for m in range(4):
                        g0 = gi * 512 + m * 128
                        ps = p1.tile([128, BS], F32)
                        for k in range(4):
                            nc.tensor.matmul(ps[:], wih_t[:, k, g0:g0 + 128],
                                             xT[:, k, :], start=(k == 0), stop=False)
                        for k in range(4):
                            nc.tensor.matmul(ps[:], whh_t[:, k, g0:g0 + 128],
                                             lhT_t[:, k, :], start=False, stop=(k == 3))
                        nc.scalar.activation(dst[:, m, :], ps[:], AF.Sigmoid,
                                             bias=bias_t[:, bcol + m:bcol + m + 1])

                # n gate: tanh(gi_n + r * gh_n + b)
                for m in range(4):
                    g0 = 2 * 512 + m * 128
                    ps_i = p1.tile([128, BS], F32, tag="psi")
                    ps_h = p1.tile([128, BS], F32, tag="psh")
                    for k in range(4):
                        nc.tensor.matmul(ps_i[:], wih_t[:, k, g0:g0 + 128],
                                         xT[:, k, :], start=(k == 0), stop=(k == 3))
                    for k in range(4):
                        nc.tensor.matmul(ps_h[:], whh_t[:, k, g0:g0 + 128],
                                         lhT_t[:, k, :], start=(k == 0), stop=(k == 3))
                    tmp0 = sn.tile([128, BS], F32, tag="tmp0")
                    nc.scalar.activation(tmp0[:], ps_h[:], AF.Identity,
                                         bias=bias_t[:, 26 + m:27 + m])
                    tmp = sn.tile([128, BS], F32, tag="tmp")
                    nc.vector.tensor_mul(tmp[:], r_t[:, m, :], tmp0[:])
                    tmp2 = sn.tile([128, BS], F32, tag="tmp2")
                    nc.vector.tensor_add(tmp2[:], tmp[:], ps_i[:])
                    nc.scalar.activation(n_t[:, m, :], tmp2[:], AF.Tanh,
                                         bias=bias_t[:, 12 + m:13 + m])

                # h' = n + z*(h - n);  rh = relu(h')
                for m in range(4):
                    d1 = sn.tile([128, BS], F32, tag="d1")
                    nc.vector.tensor_sub(d1[:], lhT_t[:, m, :].bitcast(F32), n_t[:, m, :])
                    d2 = sn.tile([128, BS], F32, tag="d2")
                    nc.vector.tensor_mul(d2[:], z_t[:, m, :], d1[:])
                    nc.vector.tensor_add(hT_t[:, m, :], n_t[:, m, :], d2[:])
                    nc.vector.tensor_scalar_max(rh_t[:, m, :], hT_t[:, m, :], 0.0)
                nc.sync.dma_start(split_kp(hT_d), hT_t[:])

                # proj: rnn_outT = projT.T @ rh + b   (emitted as f32r for scores)
                for m in range(4):
                    ps = p1.tile([128, BS], F32)
                    for k in range(4):
                        nc.tensor.matmul(ps[:], projw_t[:, k, m * 128:(m + 1) * 128],
                                         rh_t[:, k, :], start=(k == 0), stop=(k == 3))
                    nc.scalar.activation(rT[:, m, :], ps[:], AF.Identity,
                                         bias=bias_t[:, 16 + m:17 + m])

            # ================= scores / softmax / pooling ====================
            with ExitStack() as ctx:
                ep = ctx.enter_context(tc.tile_pool(name="e", bufs=1))
                ndp = ctx.enter_context(tc.tile_pool(name="nd", bufs=3))
                natp = ctx.enter_context(tc.tile_pool(name="nat", bufs=4))
                eselp = ctx.enter_context(tc.tile_pool(name="esel", bufs=3))
                scp = ctx.enter_context(tc.tile_pool(name="sc", bufs=2, space="PSUM"))
                tpp = ctx.enter_context(tc.tile_pool(name="tp", bufs=2, space="PSUM"))
                cxp = ctx.enter_context(tc.tile_pool(name="cx", bufs=1, space="PSUM"))
                fin = ctx.enter_context(tc.tile_pool(name="fin", bufs=1))
                p3 = ctx.enter_context(tc.tile_pool(name="p3", bufs=1, space="PSUM"))

                ctx_ps = cxp.tile([GPC, H], F32)
                e_tiles = [[ep.tile([128, 2 * CHUNK], F32, tag=f"e{h}_{ci}",
                                    name=f"e{h}_{ci}")
                            for ci in range(NCHUNK // 2)] for h in range(2)]

                nat_r = nat_d.rearrange("(a k p) f -> a p k f", k=4, p=128)
                sub = 0
                for ci in range(NCHUNK):
                    nd = ndp.tile([128, 4, CHUNK], F32R, tag="nd")
                    nc.sync.dma_start(
                        nd[:], split_kp(ndT_d)[:, :, ci * CHUNK:(ci + 1) * CHUNK])
                    nats = []
                    for g2 in range(CHUNK // 512):
                        natt = natp.tile([128, 4, H], BF16, tag="nat")
                        nc.sync.dma_start(natt[:], nat_r[ci * (CHUNK // 512) + g2])
                        nats.append(natt)
                    for half in range(2):
                        ps = scp.tile([128, CHUNK], F32)
                        for s2 in range(CHUNK // MMN):
                            for k in range(4):
                                nc.tensor.matmul(
                                    ps[:, s2 * MMN:(s2 + 1) * MMN],
                                    rT[:, k, half * 128:(half + 1) * 128],
                                    nd[:, k, s2 * MMN:(s2 + 1) * MMN],
                                    start=(k == 0), stop=(k == 3))
                        et = e_tiles[half][ci // 2]
                        e0 = (ci % 2) * CHUNK
                        for s in range(4):
                            seg = ci * 4 + s
                            nc.scalar.activation(
                                et[:, e0 + s * L:e0 + (s + 1) * L],
                                ps[:, s * L:(s + 1) * L],
                                AF.Exp, accum_out=den[half][:, seg:seg + 1])
                        if half == 0:
                            for j in range(NSUB):
                                gidx = sub + j
                                pst = tpp.tile([128, 32], F32)
                                nc.tensor.transpose(
                                    pst[:], et[0:32, e0 + j * 128:e0 + (j + 1) * 128],
                                    i32_t[:])
                                esel = eselp.tile([128, GPC], BF16, tag="esel")
                                nc.vector.tensor_mul(
                                    esel[:], oneh_t[:, gidx, :], pst[:])
                                nc.tensor.matmul(
                                    ctx_ps[:], esel[:], nats[j // 4][:, j % 4, :],
                                    start=(gidx == 0), stop=(gidx == NPAD // 128 - 1),
                                    skip_group_check=True)
                    # per-chunk: finalize denominators of these 4 segments,
                    # normalize in place, and drain finished 2-chunk tiles
                    for half in range(2):
                        s0 = ci * 4
                        nc.vector.tensor_sub(den[half][:, s0:s0 + 4],
                                             den[half][:, s0:s0 + 4],
                                             padc_t[:, s0:s0 + 4])
                        nc.vector.reciprocal(rcp[half][:, s0:s0 + 4],
                                             den[half][:, s0:s0 + 4])
                        et = e_tiles[half][ci // 2]
                        e0 = (ci % 2) * CHUNK
                        for s in range(4):
                            seg = s0 + s
                            nc.vector.tensor_scalar_mul(
                                et[:, e0 + s * L:e0 + (s + 1) * L],
                                et[:, e0 + s * L:e0 + (s + 1) * L],
                                rcp[half][:, seg:seg + 1])
                        if ci % 2 == 1:
                            ci2 = ci // 2
                            nc.sync.dma_start(
                                attn_d[half * 128:(half + 1) * 128,
                                       ci2 * 2 * CHUNK:(ci2 + 1) * 2 * CHUNK], et[:])
                    sub += NSUB

                # context = diag(1/den) * ctx_ps
                dtmp = fin.tile([32, 32], F32, tag="dtmp")
                nc.vector.tensor_mul(dtmp[:], rcp[0][0:32, :], i32_t[:])
                rdiag = fin.tile([32, 1], F32, tag="rdiag")
                nc.vector.tensor_reduce(rdiag[:], dtmp[:], axis=AX.X, op=ALU.add)
                ctx_sb = fin.tile([GPC, H], F32, tag="ctx")
                nc.vector.tensor_scalar_mul(ctx_sb[:], ctx_ps[:], rdiag[:])
                for m in range(4):
                    pst = tpp.tile([128, 32], F32)
                    nc.tensor.transpose(
                        pst[:], ctx_sb[:, m * 128:(m + 1) * 128], i32_t[:])
                    nc.vector.tensor_copy(ctxT[:, m, :], pst[:])

                # compress: relu(cw.T @ [rnn_out; context; x] + b)
                coT = fin.tile([128, 4, GPC], F32, tag="coT")
                for m in range(4):
                    ps = p3.tile([128, GPC], F32)
                    for kc in range(12):
                        if kc < 4:
                            rhs = rT[:, kc, 0:GPC].bitcast(F32)
                        elif kc < 8:
                            rhs = ctxT[:, kc - 4, :]
                        else:
                            rhs = xT[:, kc - 8, 0:GPC].bitcast(F32)
                        nc.tensor.matmul(ps[:], cw_t[:, kc, m * 128:(m + 1) * 128],
                                         rhs, start=(kc == 0), stop=(kc == 11))
                    nc.scalar.activation(coT[:, m, :], ps[:], AF.Relu,
                                         bias=bias_t[:, 20 + m:21 + m])

                # out layer
                ot = fin.tile([128, 2, GPC], F32, tag="ot")
                for m in range(2):
                    ps = p3.tile([128, GPC], F32)
                    for k in range(4):
                        nc.tensor.matmul(ps[:], ow_t[:, k, m * 128:(m + 1) * 128],
                                         coT[:, k, :], start=(k == 0), stop=(k == 3))
                    nc.scalar.activation(ot[:, m, :], ps[:], AF.Identity,
                                         bias=bias_t[:, 24 + m:25 + m])
                nc.sync.dma_start(
                    outT_d.rearrange("(m p) j -> p m j", p=128), ot[:])

    nc.compile()
    return nc


def _prep_host(inputs):
    """Shard + pad + transpose the full inputs into per-core input maps."""
    batch = np.asarray(inputs["batch"]).astype(np.int64)
    nodes = np.ascontiguousarray(np.asarray(inputs["nodes"], np.float32))
    cnt = np.bincount(batch, minlength=BS)
    assert cnt.sum() == batch.shape[0]
    starts = np.zeros(BS + 1, np.int64)
    np.cumsum(cnt, out=starts[1:])
    L = max(256, 128 * int(np.ceil(cnt.max() / 128)))
    NPAD = GPC * L

    inT = np.ascontiguousarray(np.asarray(inputs["input_seq"], np.float32)[0].T)
    lhT = np.ascontiguousarray(np.asarray(inputs["last_hidden"], np.float32)[0].T)
    W = {k: np.ascontiguousarray(np.asarray(inputs[k], np.float32).T)
         for k in ("enc_w", "W_ih", "W_hh", "proj_w", "compress_w", "out_w")}
    b_ih = np.asarray(inputs["b_ih"], np.float32)
    b_hh = np.asarray(inputs["b_hh"], np.float32)
    bias = np.zeros((128, 30), np.float32)
    bias[:, 0:4] = np.asarray(inputs["enc_b"], np.float32).reshape(4, 128).T
    bias[:, 4:12] = (b_ih + b_hh)[:1024].reshape(8, 128).T
    bias[:, 12:16] = b_ih[1024:].reshape(4, 128).T
    bias[:, 26:30] = b_hh[1024:].reshape(4, 128).T
    bias[:, 16:20] = np.asarray(inputs["proj_b"], np.float32).reshape(4, 128).T
    bias[:, 20:24] = np.asarray(inputs["compress_b"], np.float32).reshape(4, 128).T
    bias[:, 24:26] = np.asarray(inputs["out_b"], np.float32).reshape(2, 128).T
    i32 = np.eye(32, dtype=np.float32)

    in_maps = []
    meta = []
    for c in range(NCORES):
        ndT = np.zeros((H, NPAD), np.float32)
        nat = np.zeros((NPAD, H), np.float32)
        oneh = np.zeros((NPAD, GPC), np.float32)
        padc = np.zeros((128, GPC), np.float32)
        segs = []
        for j in range(GPC):
            g = c * GPC + j
            s, e = starts[g], starts[g + 1]
            n = e - s
            ndT[:, j * L:j * L + n] = nodes[s:e].T
            nat[j * L:j * L + n] = nodes[s:e]
            oneh[j * L:j * L + n, j] = 1.0
            padc[:, j] = L - n
            segs.append((s, e, n))
        meta.append(segs)
        in_maps.append({
            "ndT": _round_f32r(ndT),
            "nat": nat.astype(ml_dtypes.bfloat16),
            "oneh": oneh,
            "inT": _round_f32r(np.roll(inT, -GPC * c, axis=1)),
            "lhT": _round_f32r(np.roll(lhT, -GPC * c, axis=1)),
            "encw": _round_f32r(W["enc_w"]), "wih": _round_f32r(W["W_ih"]),
            "whh": _round_f32r(W["W_hh"]), "projw": _round_f32r(W["proj_w"]),
            "cw": W["compress_w"], "ow": W["out_w"],
            "bias": bias, "padc": padc, "i32": i32,
        })
    return in_maps, meta, L


def kernel(**inputs):
    from concourse.bass_utils import run_bass_kernel_spmd

    in_maps, meta, L = _prep_host(inputs)
    if L not in _PROG_CACHE:
        _PROG_CACHE[L] = _build_program(L)
    nc = _PROG_CACHE[L]

    res = run_bass_kernel_spmd(nc, in_maps, list(range(NCORES)))

    attn = np.empty((BS, N_NODES), np.float32)
    out = np.empty((BS, OUT), np.float32)
    for c in range(NCORES):
        r = res.results[c]
        ap = np.roll(r["attn"], GPC * c, axis=0)
        for j, (s, e, n) in enumerate(meta[c]):
            attn[:, s:e] = ap[:, j * L:j * L + n]
        out[c * GPC:(c + 1) * GPC] = r["outT"].T
    hidden = res.results[0]["hT"].T[None]
    return out, np.ascontiguousarray(hidden), attn


# revision 10
# speedup vs baseline: 21.7261x; 1.4588x over previous
"""Trainium2 Bass kernel for nn_LuongAttnDecoderRNN_79474074845199.

Strategy (8-core SPMD, pure data parallel per sharding hint):
  - Graphs (bs=256) are sharded 32/core; batch is sorted so each core owns a
    contiguous node range. Each graph's nodes are padded to a uniform L so one
    compiled program serves all cores (segment boundaries are compile-time).
  - Per-core batch order is ROTATED so that each core's own 32 graphs sit at
    rows 0..31 of the first half-tile; this keeps the program identical across
    cores (host un-rotates outputs).
  - The small dense net (enc -> GRU -> proj, compress -> out) runs replicated
    on every core in transposed space (activations stored as [feature, batch]).
  - scores = rnn_out @ nodes^T computed as [b, node] tiles on the PE
    (lhsT = rnn_outT, rhs = host-pre-transposed nodesT), softmax without
    max-subtraction (scores are bounded ~|30|), denominators via ACT exp
    accum_out per segment, exact zero-pad correction by subtracting pad counts.
  - Context pooling: PE-transpose of each core's own 32 e-rows -> [node, 32]
    chunks, masked by a host one-hot, then bf16 matmul against natural-layout
    bf16 nodes accumulating [32, 512] in PSUM; scaled by 1/denom diag.
"""

import numpy as np
import ml_dtypes

BS, N_NODES, IN, H, OUT = 256, 51200, 512, 512, 256
NCORES = 8
GPC = BS // NCORES  # graphs per core = 32

_PROG_CACHE = {}


def _round_f32r(x):
    """Round-to-nearest-even fp32 -> fp32r (11-bit mantissa, low 12 bits zero)."""
    u = np.ascontiguousarray(x, np.float32).view(np.uint32).astype(np.uint64)
    u = u + 0x7FF + ((u >> 12) & 1)
    u = (u & 0xFFFFF000).astype(np.uint32)
    return u.view(np.float32)


def _build_program(L, repeat=1):
    import concourse.bacc as bacc
    import concourse.tile as tile
    import concourse.mybir as mybir
    from contextlib import ExitStack

    F32 = mybir.dt.float32
    F32R = mybir.dt.float32r
    BF16 = mybir.dt.bfloat16
    AF = mybir.ActivationFunctionType
    ALU = mybir.AluOpType
    AX = mybir.AxisListType

    NPAD = GPC * L
    CHUNK = 4 * L                 # per-PSUM-tile columns (4 segments)
    NCHUNK = NPAD // CHUNK        # chunks per half
    NSUB = CHUNK // 128           # 128-node subchunks per chunk
    MMN = 512                     # matmul free-dim granularity
    assert CHUNK % MMN == 0

    nc = bacc.Bacc("TRN2", target_bir_lowering=False, debug=False)

    # ---- DRAM I/O ----
    ndT_d = nc.dram_tensor("ndT", [H, NPAD], F32R, kind="ExternalInput").ap()
    nat_d = nc.dram_tensor("nat", [NPAD, H], BF16, kind="ExternalInput").ap()
    oneh_d = nc.dram_tensor("oneh", [NPAD, GPC], F32, kind="ExternalInput").ap()
    inT_d = nc.dram_tensor("inT", [IN, BS], F32R, kind="ExternalInput").ap()
    lhT_d = nc.dram_tensor("lhT", [H, BS], F32R, kind="ExternalInput").ap()
    encw_d = nc.dram_tensor("encw", [IN, H], F32R, kind="ExternalInput").ap()
    wih_d = nc.dram_tensor("wih", [H, 3 * H], F32R, kind="ExternalInput").ap()
    whh_d = nc.dram_tensor("whh", [H, 3 * H], F32R, kind="ExternalInput").ap()
    projw_d = nc.dram_tensor("projw", [H, H], F32R, kind="ExternalInput").ap()
    cw_d = nc.dram_tensor("cw", [3 * H, H], F32, kind="ExternalInput").ap()
    ow_d = nc.dram_tensor("ow", [H, OUT], F32, kind="ExternalInput").ap()
    bias_d = nc.dram_tensor("bias", [128, 30], F32, kind="ExternalInput").ap()
    padc_d = nc.dram_tensor("padc", [128, GPC], F32, kind="ExternalInput").ap()
    i32_d = nc.dram_tensor("i32", [32, 32], F32, kind="ExternalInput").ap()

    attn_d = nc.dram_tensor("attn", [BS, NPAD], BF16, kind="ExternalOutput").ap()
    hT_d = nc.dram_tensor("hT", [H, BS], F32, kind="ExternalOutput").ap()
    outT_d = nc.dram_tensor("outT", [OUT, GPC], F32, kind="ExternalOutput").ap()

    split_kp = lambda ap: ap.rearrange("(k p) n -> p k n", p=128)

    with tile.TileContext(nc) as tc:
      for _rep in range(repeat):
        with ExitStack() as octx:
            persist = octx.enter_context(tc.tile_pool(name="persist", bufs=1))
            bias_t = persist.tile([128, 30], F32, tag="bias")
            i32_t = persist.tile([32, 32], F32, tag="i32")
            padc_t = persist.tile([128, GPC], F32, tag="padc")
            oneh_t = persist.tile([128, NPAD // 128, GPC], F32, tag="oneh")
            xT = persist.tile([128, 4, BS], F32R, tag="xT")
            rT = persist.tile([128, 4, BS], F32R, tag="rT")
            cw_t = persist.tile([128, 12, H], F32, tag="cw")
            ow_t = persist.tile([128, 4, OUT], F32, tag="ow")
            den = [persist.tile([128, GPC], F32, tag=f"den{h}", name=f"den{h}")
                   for h in range(2)]
            rcp = [persist.tile([128, GPC], F32, tag=f"rcp{h}", name=f"rcp{h}")
                   for h in range(2)]
            ctxT = persist.tile([128, 4, GPC], F32, tag="ctxT")

            nc.sync.dma_start(bias_t[:], bias_d)
            nc.sync.dma_start(i32_t[:], i32_d)
            nc.sync.dma_start(padc_t[:], padc_d)
            nc.sync.dma_start(oneh_t[:], oneh_d.rearrange("(s p) j -> p s j", p=128))
            nc.sync.dma_start(cw_t[:], split_kp(cw_d))
            nc.sync.dma_start(ow_t[:], split_kp(ow_d))

            # ================= small net (replicated, transposed space) =====
            with ExitStack() as ctx:
                wp = ctx.enter_context(tc.tile_pool(name="wts", bufs=1))
                sn = ctx.enter_context(tc.tile_pool(name="sn", bufs=1))
                p1 = ctx.enter_context(tc.tile_pool(name="p1", bufs=2, space="PSUM"))

                encw_t = wp.tile([128, 4, H], F32R, tag="encw")
                wih_t = wp.tile([128, 4, 3 * H], F32R, tag="wih")
                whh_t = wp.tile([128, 4, 3 * H], F32R, tag="whh")
                projw_t = wp.tile([128, 4, H], F32R, tag="projw")
                inT_t = wp.tile([128, 4, BS], F32R, tag="inT")
                lhT_t = wp.tile([128, 4, BS], F32R, tag="lhT")
                nc.sync.dma_start(encw_t[:], split_kp(encw_d))
                nc.sync.dma_start(wih_t[:], split_kp(wih_d))
                nc.sync.dma_start(whh_t[:], split_kp(whh_d))
                nc.sync.dma_start(projw_t[:], split_kp(projw_d))
                nc.sync.dma_start(inT_t[:], split_kp(inT_d))
                nc.sync.dma_start(lhT_t[:], split_kp(lhT_d))

                r_t = sn.tile([128, 4, BS], F32, tag="r")
                z_t = sn.tile([128, 4, BS], F32, tag="z")
                n_t = sn.tile([128, 4, BS], F32, tag="n")
                hT_t = sn.tile([128, 4, BS], F32, tag="h")
                rh_t = sn.tile([128, 4, BS], F32R, tag="rh")

                # enc: xT = relu(enc_wT.T @ inT + b)
                for m in range(4):
                    ps = p1.tile([128, BS], F32)
                    for k in range(4):
                        nc.tensor.matmul(ps[:], encw_t[:, k, m * 128:(m + 1) * 128],
                                         inT_t[:, k, :], start=(k == 0), stop=(k == 3))
                    nc.scalar.activation(xT[:, m, :], ps[:], AF.Relu,
                                         bias=bias_t[:, m:m + 1])

                # r and z gates: sigmoid(W_ih.x + W_hh.h + b)  (PE accumulates both)
                for gi, bcol, dst in ((0, 4, r_t), (1, 8, z_t)):
                    for m in range(4):
                        g0 = gi * 512 + m * 128
                        ps = p1.tile([128, BS], F32)
                        for k in range(4):
                            nc.tensor.matmul(ps[:], wih_t[:, k, g0:g0 + 128],
                                             xT[:, k, :], start=(k == 0), stop=False)
                        for k in range(4):
                            nc.tensor.matmul(ps[:], whh_t[:, k, g0:g0 + 128],
                                             lhT_t[:, k, :], start=False, stop=(k == 3))
                        nc.scalar.activation(dst[:, m, :], ps[:], AF.Sigmoid,
                                             bias=bias_t[:, bcol + m:bcol + m + 1])

                # n gate: tanh(gi_n + r * gh_n + b)
                for m in range(4):
                    g0 = 2 * 512 + m * 128
                    ps_i = p1.tile([128, BS], F32, tag="psi")
                    ps_h = p1.tile([128, BS], F32, tag="psh")
                    for k in range(4):
                        nc.tensor.matmul(ps_i[:], wih_t[:, k, g0:g0 + 128],
                                         xT[:, k, :], start=(k == 0), stop=(k == 3))
                    for k in range(4):
                        nc.tensor.matmul(ps_h[:], whh_t[:, k, g0:g0 + 128],
                                         lhT_t[:, k, :], start=(k == 0), stop=(k == 3))
                    tmp0 = sn.tile([128, BS], F32, tag="tmp0")
                    nc.scalar.activation(tmp0[:], ps_h[:], AF.Identity,
                                         bias=bias_t[:, 26 + m:27 + m])
                    tmp = sn.tile([128, BS], F32, tag="tmp")
                    nc.vector.tensor_mul(tmp[:], r_t[:, m, :], tmp0[:])
                    tmp2 = sn.tile([128, BS], F32, tag="tmp2")
                    nc.vector.tensor_add(tmp2[:], tmp[:], ps_i[:])
                    nc.scalar.activation(n_t[:, m, :], tmp2[:], AF.Tanh,
                                         bias=bias_t[:, 12 + m:13 + m])

                # h' = n + z*(h - n);  rh = relu(h')
                for m in range(4):
                    d1 = sn.tile([128, BS], F32, tag="d1")
                    nc.vector.tensor_sub(d1[:], lhT_t[:, m, :].bitcast(F32), n_t[:, m, :])
                    d2 = sn.tile([128, BS], F32, tag="d2")
                    nc.vector.tensor_mul(d2[:], z_t[:, m, :], d1[:])
                    nc.vector.tensor_add(hT_t[:, m, :], n_t[:, m, :], d2[:])
                    nc.vector.tensor_scalar_max(rh_t[:, m, :], hT_t[:, m, :], 0.0)
                nc.sync.dma_start(split_kp(hT_d), hT_t[:])

                # proj: rnn_outT = projT.T @ rh + b   (emitted as f32r for scores)
                for m in range(4):
                    ps = p1.tile([128, BS], F32)
                    for k in range(4):
                        nc.tensor.matmul(ps[:], projw_t[:, k, m * 128:(m + 1) * 128],
                                         rh_t[:, k, :], start=(k == 0), stop=(k == 3))
                    nc.scalar.activation(rT[:, m, :], ps[:], AF.Identity,
                                         bias=bias_t[:, 16 + m:17 + m])

            # ================= scores / softmax / pooling ====================
            with ExitStack() as ctx:
                ep = ctx.enter_context(tc.tile_pool(name="e", bufs=4))
                wp16 = ctx.enter_context(tc.tile_pool(name="w16", bufs=3))
                ndp = ctx.enter_context(tc.tile_pool(name="nd", bufs=3))
                natp = ctx.enter_context(tc.tile_pool(name="nat", bufs=3))
                eselp = ctx.enter_context(tc.tile_pool(name="esel", bufs=3))
                scp = ctx.enter_context(tc.tile_pool(name="sc", bufs=2, space="PSUM"))
                tpp = ctx.enter_context(tc.tile_pool(name="tp", bufs=2, space="PSUM"))
                cxp = ctx.enter_context(tc.tile_pool(name="cx", bufs=1, space="PSUM"))
                fin = ctx.enter_context(tc.tile_pool(name="fin", bufs=1))
                p3 = ctx.enter_context(tc.tile_pool(name="p3", bufs=1, space="PSUM"))

                ctx_ps = cxp.tile([GPC, H], F32)
                nat_r = nat_d.rearrange("(a k p) f -> a p k f", k=NSUB, p=128)
                cur_e = [None, None]
                cur_w = [None, None]
                sub = 0
                for ci in range(NCHUNK):
                    nd = ndp.tile([128, 4, CHUNK], F32R, tag="nd")
                    nc.sync.dma_start(
                        nd[:], split_kp(ndT_d)[:, :, ci * CHUNK:(ci + 1) * CHUNK])
                    natt = natp.tile([128, NSUB, H], BF16, tag="nat")
                    nc.sync.dma_start(natt[:], nat_r[ci])
                    for half in range(2):
                        ps = scp.tile([128, CHUNK], F32)
                        for s2 in range(CHUNK // MMN):
                            for k in range(4):
                                nc.tensor.matmul(
                                    ps[:, s2 * MMN:(s2 + 1) * MMN],
                                    rT[:, k, half * 128:(half + 1) * 128],
                                    nd[:, k, s2 * MMN:(s2 + 1) * MMN],
                                    start=(k == 0), stop=(k == 3))
                        if ci % 2 == 0:
                            cur_e[half] = ep.tile([128, 2 * CHUNK], F32, tag="e",
                                                  name=f"e_{half}_{ci}")
                            cur_w[half] = wp16.tile([128, 2 * CHUNK], BF16,
                                                    tag="w16", name=f"w_{half}_{ci}")
                        et = cur_e[half]
                        e0 = (ci % 2) * CHUNK
                        for s in range(4):
                            seg = ci * 4 + s
                            nc.scalar.activation(
                                et[:, e0 + s * L:e0 + (s + 1) * L],
                                ps[:, s * L:(s + 1) * L],
                                AF.Exp, accum_out=den[half][:, seg:seg + 1])
                        if half == 0:
                            for j in range(NSUB):
                                gidx = sub + j
                                pst = tpp.tile([128, 32], F32)
                                nc.tensor.transpose(
                                    pst[:], et[0:32, e0 + j * 128:e0 + (j + 1) * 128],
                                    i32_t[:])
                                esel = eselp.tile([128, GPC], BF16, tag="esel")
                                nc.vector.tensor_mul(
                                    esel[:], oneh_t[:, gidx, :], pst[:])
                                nc.tensor.matmul(
                                    ctx_ps[:], esel[:], natt[:, j, :],
                                    start=(gidx == 0), stop=(gidx == NPAD // 128 - 1),
                                    skip_group_check=True)
                    # per-chunk: finalize these 4 segments' denominators and
                    # normalize into the bf16 store tile; drain per tile-pair
                    for half in range(2):
                        s0 = ci * 4
                        nc.vector.tensor_sub(den[half][:, s0:s0 + 4],
                                             den[half][:, s0:s0 + 4],
                                             padc_t[:, s0:s0 + 4])
                        nc.vector.reciprocal(rcp[half][:, s0:s0 + 4],
                                             den[half][:, s0:s0 + 4])
                        et = cur_e[half]
                        w16 = cur_w[half]
                        e0 = (ci % 2) * CHUNK
                        for s in range(4):
                            seg = s0 + s
                            nc.vector.tensor_scalar_mul(
                                w16[:, e0 + s * L:e0 + (s + 1) * L],
                                et[:, e0 + s * L:e0 + (s + 1) * L],
                                rcp[half][:, seg:seg + 1])
                        if ci % 2 == 1:
                            ci2 = ci // 2
                            nc.sync.dma_start(
                                attn_d[half * 128:(half + 1) * 128,
                                       ci2 * 2 * CHUNK:(ci2 + 1) * 2 * CHUNK],
                                w16[:])
                    sub += NSUB

                # context = diag(1/den) * ctx_ps
                dtmp = fin.tile([32, 32], F32, tag="dtmp")
                nc.vector.tensor_mul(dtmp[:], rcp[0][0:32, :], i32_t[:])
                rdiag = fin.tile([32, 1], F32, tag="rdiag")
                nc.vector.tensor_reduce(rdiag[:], dtmp[:], axis=AX.X, op=ALU.add)
                ctx_sb = fin.tile([GPC, H], F32, tag="ctx")
                nc.vector.tensor_scalar_mul(ctx_sb[:], ctx_ps[:], rdiag[:])
                for m in range(4):
                    pst = tpp.tile([128, 32], F32)
                    nc.tensor.transpose(
                        pst[:], ctx_sb[:, m * 128:(m + 1) * 128], i32_t[:])
                    nc.vector.tensor_copy(ctxT[:, m, :], pst[:])

                # compress: relu(cw.T @ [rnn_out; context; x] + b)
                coT = fin.tile([128, 4, GPC], F32, tag="coT")
                for m in range(4):
                    ps = p3.tile([128, GPC], F32)
                    for kc in range(12):
                        if kc < 4:
                            rhs = rT[:, kc, 0:GPC].bitcast(F32)
                        elif kc < 8:
                            rhs = ctxT[:, kc - 4, :]
                        else:
                            rhs = xT[:, kc - 8, 0:GPC].bitcast(F32)
                        nc.tensor.matmul(ps[:], cw_t[:, kc, m * 128:(m + 1) * 128],
                                         rhs, start=(kc == 0), stop=(kc == 11))
                    nc.scalar.activation(coT[:, m, :], ps[:], AF.Relu,
                                         bias=bias_t[:, 20 + m:21 + m])

                # out layer
                ot = fin.tile([128, 2, GPC], F32, tag="ot")
                for m in range(2):
                    ps = p3.tile([128, GPC], F32)
                    for k in range(4):
                        nc.tensor.matmul(ps[:], ow_t[:, k, m * 128:(m + 1) * 128],
                                         coT[:, k, :], start=(k == 0), stop=(k == 3))
                    nc.scalar.activation(ot[:, m, :], ps[:], AF.Identity,
                                         bias=bias_t[:, 24 + m:25 + m])
                nc.sync.dma_start(
                    outT_d.rearrange("(m p) j -> p m j", p=128), ot[:])

    nc.compile()
    return nc


def _prep_host(inputs):
    """Shard + pad + transpose the full inputs into per-core input maps."""
    batch = np.asarray(inputs["batch"]).astype(np.int64)
    nodes = np.ascontiguousarray(np.asarray(inputs["nodes"], np.float32))
    cnt = np.bincount(batch, minlength=BS)
    assert cnt.sum() == batch.shape[0]
    starts = np.zeros(BS + 1, np.int64)
    np.cumsum(cnt, out=starts[1:])
    L = max(256, 128 * int(np.ceil(cnt.max() / 128)))
    NPAD = GPC * L

    inT = np.ascontiguousarray(np.asarray(inputs["input_seq"], np.float32)[0].T)
    lhT = np.ascontiguousarray(np.asarray(inputs["last_hidden"], np.float32)[0].T)
    W = {k: np.ascontiguousarray(np.asarray(inputs[k], np.float32).T)
         for k in ("enc_w", "W_ih", "W_hh", "proj_w", "compress_w", "out_w")}
    b_ih = np.asarray(inputs["b_ih"], np.float32)
    b_hh = np.asarray(inputs["b_hh"], np.float32)
    bias = np.zeros((128, 30), np.float32)
    bias[:, 0:4] = np.asarray(inputs["enc_b"], np.float32).reshape(4, 128).T
    bias[:, 4:12] = (b_ih + b_hh)[:1024].reshape(8, 128).T
    bias[:, 12:16] = b_ih[1024:].reshape(4, 128).T
    bias[:, 26:30] = b_hh[1024:].reshape(4, 128).T
    bias[:, 16:20] = np.asarray(inputs["proj_b"], np.float32).reshape(4, 128).T
    bias[:, 20:24] = np.asarray(inputs["compress_b"], np.float32).reshape(4, 128).T
    bias[:, 24:26] = np.asarray(inputs["out_b"], np.float32).reshape(2, 128).T
    i32 = np.eye(32, dtype=np.float32)

    in_maps = []
    meta = []
    for c in range(NCORES):
        ndT = np.zeros((H, NPAD), np.float32)
        nat = np.zeros((NPAD, H), np.float32)
        oneh = np.zeros((NPAD, GPC), np.float32)
        padc = np.zeros((128, GPC), np.float32)
        segs = []
        for j in range(GPC):
            g = c * GPC + j
            s, e = starts[g], starts[g + 1]
            n = e - s
            ndT[:, j * L:j * L + n] = nodes[s:e].T
            nat[j * L:j * L + n] = nodes[s:e]
            oneh[j * L:j * L + n, j] = 1.0
            padc[:, j] = L - n
            segs.append((s, e, n))
        meta.append(segs)
        in_maps.append({
            "ndT": _round_f32r(ndT),
            "nat": nat.astype(ml_dtypes.bfloat16),
            "oneh": oneh,
            "inT": _round_f32r(np.roll(inT, -GPC * c, axis=1)),
            "lhT": _round_f32r(np.roll(lhT, -GPC * c, axis=1)),
            "encw": _round_f32r(W["enc_w"]), "wih": _round_f32r(W["W_ih"]),
            "whh": _round_f32r(W["W_hh"]), "projw": _round_f32r(W["proj_w"]),
            "cw": W["compress_w"], "ow": W["out_w"],
            "bias": bias, "padc": padc, "i32": i32,
        })
    return in_maps, meta, L


def kernel(**inputs):
    from concourse.bass_utils import run_bass_kernel_spmd

    in_maps, meta, L = _prep_host(inputs)
    if L not in _PROG_CACHE:
        _PROG_CACHE[L] = _build_program(L)
    nc = _PROG_CACHE[L]

    res = run_bass_kernel_spmd(nc, in_maps, list(range(NCORES)))

    attn = np.empty((BS, N_NODES), np.float32)
    out = np.empty((BS, OUT), np.float32)
    for c in range(NCORES):
        r = res.results[c]
        ap = np.roll(np.asarray(r["attn"]).astype(np.float32), GPC * c, axis=0)
        for j, (s, e, n) in enumerate(meta[c]):
            attn[:, s:e] = ap[:, j * L:j * L + n]
        out[c * GPC:(c + 1) * GPC] = r["outT"].T
    hidden = res.results[0]["hT"].T[None]
    return out, np.ascontiguousarray(hidden), attn
